# revision 15
# baseline (speedup 1.0000x reference)
"""3-layer GAT (heads=1) + global mean pool on 8 Trainium2 NeuronCores — v7.

Keeps v6's compute pipeline (host-precomputed fp8 one-hots in both
orientations streamed on Sync HWDGE; ad[dst] via PE matmuls over ohT;
batched s2/leaky on DVE, exp on ACT; mee = oh*ee broadcast-mult as the main
matmul lhsT; psum->SBUF copies on ACT; in-degree-balanced windows) and adds:

- dma_gather calls round-robin over 4 SWDGE queues (num_swdge_queues=4):
  the gather cadence is per-queue serialized, 4 queues give ~3.7x.
- phase A of layer L+1 is fused into layer L's edge loop (per window, right
  after the epilogue), and each AllGather is split into two window-halves
  with layer-parity double buffering of the table: AG(first half) fires
  mid-edge-loop and overlaps the remaining compute; only AG(second half)
  plus pipeline refill remains at the layer boundary.
- the int16 gather-index split is by window half (table A = windows 0..25
  of every core, table B = 26..51), which is exactly the AllGather split.
"""
import os
import sys
import time

import numpy as np

for p in ("/root/.axon_site", "/root/.axon_site/_ro/trn_rl_repo",
          "/root/.axon_site/_ro/pypackages", "/opt/trn_rl_repo", "/opt/pypackages"):
    if os.path.isdir(p) and p not in sys.path:
        sys.path.append(p)

from contextlib import ExitStack

import concourse.bass as bass
import concourse.mybir as mybir
import concourse.tile as tile
from concourse import bacc
from concourse.bass_utils import run_bass_kernel_spmd

N_NODES = 50000
N_GRAPHS = 512
NEG_SLOPE = 0.2
EPS = 1e-16
NC = 8
P = 128
D = 128
S_PAD = 6656            # padded per-core node slice (52 windows of 128)
NWIN = S_PAD // P
WSPL = 26               # window split: table A = windows [0,26), B = [26,52)
SH = WSPL * P           # rows per half-shard (3328)
TABR = NC * SH          # rows per half-table (26624, int16-addressable)
TW = 256                # fat table row stride (bf16): [h|as|ad|one|pad]
HW = 132                # meaningful table columns
GW = 131                # matmul rhs width (h | as | ad | one)
VOCAB = 32000
GMAX = 56               # max chunks (A+B) per gather group
SUB = 8                 # max chunks per dma_gather instruction (1024 idxs)
HB_BUFS = 4             # h_bf ring buffers (pads memset once per buffer)

last_exec_time_ns = None
_COMPILED = {}


# ---------------------------------------------------------------- host prep
def _balance_windows(in_deg, S_c):
    """LPT bin-packing of S_c nodes into NWIN bins of P by in-degree.
    Returns perm: new position -> local node index (padded with -1)."""
    order = np.argsort(-in_deg, kind="stable")
    loads = np.zeros(NWIN, np.int64)
    counts = np.zeros(NWIN, np.int64)
    assign = np.empty(S_c, np.int64)
    for n in order:
        cand = np.where(counts < P)[0]
        w = cand[np.argmin(loads[cand])]
        assign[n] = w
        loads[w] += in_deg[n]
        counts[w] += 1
    perm = np.full(S_PAD, -1, np.int64)
    pos = np.zeros(NWIN, np.int64)
    for n in range(S_c):
        w = assign[n]
        perm[w * P + pos[w]] = n
        pos[w] += 1
    out = np.full(S_PAD, -1, np.int64)
    for w in range(NWIN):
        vals = perm[w * P:(w + 1) * P]
        vals = vals[vals >= 0]
        out[w * P:w * P + len(vals)] = vals
    return out


def _prep(node_ids, edge_index, batch):
    node_ids = node_ids.astype(np.int64)
    edge_index = edge_index.astype(np.int64)
    batch = batch.astype(np.int64)

    g_start = np.searchsorted(batch, np.arange(N_GRAPHS + 1))
    target = N_NODES / NC
    bounds = [0]
    for c in range(1, NC):
        want = c * target
        gi = np.searchsorted(g_start, want)
        cand = [g_start[max(gi - 1, 0)], g_start[min(gi, N_GRAPHS)]]
        bounds.append(int(min(cand, key=lambda v: abs(v - want))))
    bounds.append(N_NODES)
    bounds = np.array(bounds)
    assert np.all(np.diff(bounds) > 0) and np.all(np.diff(bounds) <= S_PAD)
    g_bounds = [int(batch[b]) if b < N_NODES else N_GRAPHS for b in bounds[:-1]]
    g_bounds.append(N_GRAPHS)

    loop = np.arange(N_NODES, dtype=np.int64)
    src_full = np.concatenate([edge_index[0], loop])
    dst_full = np.concatenate([edge_index[1], loop])
    src_core = np.searchsorted(bounds, src_full, side="right") - 1
    dst_core = np.searchsorted(bounds, dst_full, side="right") - 1

    # per-core window balancing permutation (new position -> local node)
    in_deg_full = np.bincount(dst_full, minlength=N_NODES)
    perms, invs = [], []
    for c in range(NC):
        S_c = int(bounds[c + 1] - bounds[c])
        perm = _balance_windows(in_deg_full[bounds[c]:bounds[c + 1]], S_c)
        inv = np.full(S_PAD, -1, np.int64)
        valid = perm >= 0
        inv[perm[valid]] = np.where(valid)[0]
        perms.append(perm)
        invs.append(inv[:S_c])

    pos_of_node = np.empty(N_NODES, np.int64)
    for c in range(NC):
        pos_of_node[bounds[c]:bounds[c + 1]] = c * S_PAD + invs[c]

    src_pad_id = pos_of_node[src_full]
    dst_loc = pos_of_node[dst_full] - dst_core * S_PAD
    # A/B table id: half by src window range, table row within half-table
    src_pos = src_pad_id % S_PAD
    src_half = (src_pos >= SH).astype(np.int64)
    src_tab = (src_pad_id // S_PAD) * SH + src_pos - src_half * SH

    cores = []
    cnt_half = np.zeros((NC, NWIN, 2), np.int64)
    for c in range(NC):
        sel = np.where(dst_core == c)[0]
        dloc = dst_loc[sel]
        tab = src_tab[sel]
        half = src_half[sel]
        w_of = dloc // P
        order = np.lexsort((tab, half, w_of))   # window, half, then src row
        dloc, tab, half, w_of = (dloc[order], tab[order],
                                 half[order], w_of[order])
        for w in range(NWIN):
            m = w_of == w
            cnt_half[c, w, 0] = np.sum(m & (half == 0))
            cnt_half[c, w, 1] = np.sum(m & (half == 1))
        cores.append(dict(dloc=dloc, tab=tab, w_of=w_of, half=half,
                          s0=int(bounds[c]), S_c=int(bounds[c + 1] - bounds[c]),
                          g0=g_bounds[c], G_c=g_bounds[c + 1] - g_bounds[c],
                          perm=perms[c]))
    assert max(c["G_c"] for c in cores) <= P

    # SPMD-identical chunk counts per (window, half)
    k_wh = np.ceil(cnt_half.max(axis=0) / P).astype(np.int64)   # [NWIN, 2]
    k_lo = [int(x) for x in k_wh[:, 0]]
    k_hi = [int(x) for x in k_wh[:, 1]]
    nch_lo, nch_hi = sum(k_lo), sum(k_hi)

    for c in cores:
        for h, nch_s, k_s, tag in ((0, nch_lo, k_lo, "lo"), (1, nch_hi, k_hi, "hi")):
            src_rel = np.zeros((nch_s, P), np.int16)      # pad -> row 0 of half
            dloc_pad = np.full((nch_s, P), -1, np.int64)
            pos = 0
            for w in range(NWIN):
                m = (c["w_of"] == w) & (c["half"] == h)
                sg = c["tab"][m]
                dl = c["dloc"][m] - w * P
                ne = sg.shape[0]
                flat = np.arange(ne)
                src_rel[pos + flat // P, flat % P] = sg.astype(np.int16)
                dloc_pad[pos + flat // P, flat % P] = dl
                pos += k_s[w]
            assert pos == nch_s
            oh = (dloc_pad[:, :, None] ==
                  np.arange(P, dtype=np.int64)[None, None, :])   # [nch, e, d]
            f8 = mybir.dt.np(mybir.dt.float8e4)
            oh_ed = np.ascontiguousarray(
                oh.transpose(1, 0, 2).reshape(P, nch_s * P)).astype(f8)
            oh_de = np.ascontiguousarray(
                oh.transpose(2, 0, 1).reshape(P, nch_s * P)).astype(f8)
            L = np.ascontiguousarray(src_rel.reshape(-1))         # [nch*128]
            wrapped = L.reshape(-1, 16).T.astype(np.int16)        # [16, nch*8]
            c[f"idx16_{tag}"] = np.ascontiguousarray(np.tile(wrapped, (8, 1)))
            c[f"oh_{tag}"] = oh_ed
            c[f"ohT_{tag}"] = oh_de

        nid_src = node_ids[c["s0"]:c["s0"] + c["S_c"]]
        nid = np.full(S_PAD, nid_src[0], np.int64)
        valid = c["perm"] >= 0
        nid[valid] = nid_src[c["perm"][valid]]
        wrapped = nid.reshape(-1, 16).T.astype(np.int16)
        c["nid16"] = np.ascontiguousarray(np.tile(wrapped, (8, 1)))

        Pm = np.zeros((S_PAD, P), np.float32)
        bb_src = batch[c["s0"]:c["s0"] + c["S_c"]] - c["g0"]
        cnts = np.bincount(bb_src, minlength=c["G_c"]).astype(np.float32)
        w8 = 1.0 / np.maximum(cnts, 1.0)
        rows_pos = np.where(valid)[0]
        bb = bb_src[c["perm"][valid]]
        Pm[rows_pos, bb] = w8[bb]
        c["p_pool"] = Pm
    return cores, k_lo, k_hi


def _groups(k_lo, k_hi):
    """Window-aligned groups with at most GMAX total chunks.
    Returns list of (w0, w1, clo0, Clo, chi0, Chi)."""
    groups = []
    w0, clo0, chi0, alo, ahi = 0, 0, 0, 0, 0
    plo, phi = 0, 0
    for w in range(NWIN):
        cw = k_lo[w] + k_hi[w]
        if (alo + ahi) and (alo + ahi + cw > GMAX):
            groups.append((w0, w, clo0, alo, chi0, ahi))
            w0, clo0, chi0, alo, ahi = w, plo, phi, 0, 0
        alo += k_lo[w]
        ahi += k_hi[w]
        plo += k_lo[w]
        phi += k_hi[w]
    if alo + ahi:
        groups.append((w0, NWIN, clo0, alo, chi0, ahi))
    return groups


# ---------------------------------------------------------------- device
def _build(k_lo, k_hi):
    key = (tuple(k_lo), tuple(k_hi))
    if key in _COMPILED:
        return _COMPILED[key]
    f32, bf16 = mybir.dt.float32, mybir.dt.bfloat16
    fp8 = mybir.dt.float8e4
    i16 = mybir.dt.int16
    nc = bacc.Bacc("TRN2", num_devices=NC, num_swdge_queues=4)
    nch_lo, nch_hi = sum(k_lo), sum(k_hi)
    groups = _groups(k_lo, k_hi)
    kmax_w = max(ka + kb for ka, kb in zip(k_lo, k_hi))
    qctr = [0]

    def next_q():
        qctr[0] += 1
        return qctr[0] % 4

    emb_d = nc.declare_dram_parameter("emb_bf", [VOCAB, D], bf16, isOutput=False)
    w_d = nc.declare_dram_parameter("w_all", [3, D, HW], bf16, isOutput=False)
    b_d = nc.declare_dram_parameter("b_bcast", [3, P, D], f32, isOutput=False)
    ident_d = nc.declare_dram_parameter("ident", [P, P], bf16, isOutput=False)
    nid_d = nc.declare_dram_parameter("nid16", [P, S_PAD // 16], i16,
                                      isOutput=False)
    ilo_d = nc.declare_dram_parameter("idx16_lo", [P, nch_lo * 8], i16,
                                      isOutput=False)
    ihi_d = nc.declare_dram_parameter("idx16_hi", [P, nch_hi * 8], i16,
                                      isOutput=False)
    ohlo_d = nc.declare_dram_parameter("oh_lo", [P, nch_lo * P], fp8,
                                       isOutput=False)
    ohhi_d = nc.declare_dram_parameter("oh_hi", [P, nch_hi * P], fp8,
                                       isOutput=False)
    ohTlo_d = nc.declare_dram_parameter("ohT_lo", [P, nch_lo * P], fp8,
                                        isOutput=False)
    ohThi_d = nc.declare_dram_parameter("ohT_hi", [P, nch_hi * P], fp8,
                                        isOutput=False)
    pp_d = nc.declare_dram_parameter("p_pool", [S_PAD, P], bf16, isOutput=False)
    out_d = nc.declare_dram_parameter("out_pool", [P, D], f32, isOutput=True)

    h_shard = [nc.dram_tensor(f"h_shard{pr}", [S_PAD, TW], bf16)
               for pr in range(2)]
    h_full_a = [nc.dram_tensor(f"h_full_a{pr}", [TABR, TW], bf16,
                               addr_space="Shared") for pr in range(2)]
    h_full_b = [nc.dram_tensor(f"h_full_b{pr}", [TABR, TW], bf16,
                               addr_space="Shared") for pr in range(2)]

    with tile.TileContext(nc) as tc, ExitStack() as ctx:
        con = ctx.enter_context(tc.tile_pool(name="con", bufs=1))
        big = ctx.enter_context(tc.tile_pool(name="big", bufs=1))
        rows_p = ctx.enter_context(tc.tile_pool(name="rows", bufs=2))
        oh_p = ctx.enter_context(tc.tile_pool(name="ohp", bufs=2))
        mee_p = ctx.enter_context(tc.tile_pool(name="meep", bufs=4))
        sml_p = ctx.enter_context(tc.tile_pool(name="sml", bufs=16))
        ups_p = ctx.enter_context(tc.tile_pool(name="ups", bufs=8))
        ph_p = ctx.enter_context(tc.tile_pool(name="ph", bufs=4))
        ps = ctx.enter_context(tc.tile_pool(name="ps", bufs=3, space="PSUM"))
        psa = ctx.enter_context(tc.tile_pool(name="psa", bufs=2, space="PSUM"))
        ps1 = ctx.enter_context(tc.tile_pool(name="ps1", bufs=1, space="PSUM"))
        ps2 = ctx.enter_context(tc.tile_pool(name="ps2", bufs=1, space="PSUM"))
        psp = ctx.enter_context(tc.tile_pool(name="psp", bufs=1, space="PSUM"))

        ident_t = con.tile([P, P], bf16)
        nc.sync.dma_start(ident_t[:], ident_d[:])
        w_t, b_t = [], []
        for li in range(3):
            wt = con.tile([P, HW], bf16, tag=f"w{li}")
            nc.sync.dma_start(wt[:], w_d[li])
            w_t.append(wt)
            bt = con.tile([P, D], f32, tag=f"b{li}")
            nc.sync.dma_start(bt[:], b_d[li])
            b_t.append(bt)
        nid_t = con.tile([P, S_PAD // 16], i16)
        nc.sync.dma_start(nid_t[:], nid_d[:])
        ilo_t = con.tile([P, nch_lo * 8], i16)
        nc.sync.dma_start(ilo_t[:], ilo_d[:])
        ihi_t = con.tile([P, nch_hi * 8], i16)
        nc.sync.dma_start(ihi_t[:], ihi_d[:])

        zero_t = con.tile([P, 1], f32, name="zero_t")
        nc.vector.memset(zero_t[:], 0.0)
        eps_t = con.tile([P, 1], f32, name="eps_t")
        nc.vector.memset(eps_t[:], EPS)
        x_sb = big.tile([P, S_PAD], bf16)       # node features, window-major
        ad_t = [big.tile([P, NWIN], bf16, name=f"ad{pr}") for pr in range(2)]

        hb_count = [0]

        def phase_a(li, w, pr):
            """h(li) for window w from x_sb -> h_shard[pr] (+ ad column)."""
            ws = slice(w * D, (w + 1) * D)
            hs3 = h_shard[pr].rearrange("(v p) f -> v p f", p=P)
            xt_ps = ps2.tile([P, P], bf16, tag="tp", name="xt_ps")
            nc.tensor.transpose(out=xt_ps[:], in_=x_sb[:, ws],
                                identity=ident_t[:])
            xt = ph_p.tile([P, P], bf16, tag="xt_sb", name="xt")
            nc.scalar.activation(xt[:], xt_ps[:],
                                 mybir.ActivationFunctionType.Copy)
            h_ps = ps1.tile([P, HW], f32, tag="hps", name="h_ps")
            nc.tensor.matmul(h_ps[:], lhsT=xt[:], rhs=w_t[li][:],
                             start=True, stop=True)
            h_bf = ph_p.tile([P, TW], bf16, tag="hbf", bufs=HB_BUFS,
                             name="h_bf")
            if hb_count[0] < HB_BUFS:
                nc.vector.memset(h_bf[:, 130:131], 1.0)
                nc.vector.memset(h_bf[:, 131:TW], 0.0)
                hb_count[0] += 1
            nc.scalar.activation(h_bf[:, 0:130], h_ps[:, 0:130],
                                 mybir.ActivationFunctionType.Copy)
            nc.scalar.activation(ad_t[pr][:, w:w + 1], h_ps[:, 129:130],
                                 mybir.ActivationFunctionType.Copy)
            nc.sync.dma_start(hs3[w, :, :], h_bf[:])

        def ag_half(pr, which):
            src = h_shard[pr][0:SH, :] if which == 0 else h_shard[pr][SH:, :]
            dst = (h_full_a if which == 0 else h_full_b)[pr]
            nc.gpsimd.collective_compute(
                "AllGather", mybir.AluOpType.bypass,
                replica_groups=[list(range(NC))],
                ins=[src], outs=[dst[:]])

        # layer-0 input: embedding gather, phase A, split AllGather
        x3g = x_sb[:].rearrange("p (w f) -> p w f", f=D)
        for g0 in range(0, NWIN, SUB):
            g1 = min(g0 + SUB, NWIN)
            nc.gpsimd.dma_gather(
                out_ap=x3g[:, g0:g1, :], in_ap=emb_d[:],
                idxs_ap=nid_t[:, g0 * 8:g1 * 8],
                num_idxs=(g1 - g0) * P, num_idxs_reg=(g1 - g0) * P,
                elem_size=D, queue_num=next_q())
        for w in range(NWIN):
            phase_a(0, w, 0)
        ag_half(0, 0)
        ag_half(0, 1)

        for li in range(3):
            pr = li % 2
            for (w0, w1, clo0, Clo, chi0, Chi) in groups:
                C = Clo + Chi
                rows_g = rows_p.tile([P, GMAX * TW], bf16, tag="rows",
                                     name="rows_g")
                rows3 = rows_g[:].rearrange("p (c f) -> p c f", f=TW)
                for g0 in range(0, Clo, SUB):
                    g1 = min(g0 + SUB, Clo)
                    nc.gpsimd.dma_gather(
                        out_ap=rows3[:, g0:g1, :], in_ap=h_full_a[pr][:],
                        idxs_ap=ilo_t[:, (clo0 + g0) * 8:(clo0 + g1) * 8],
                        num_idxs=(g1 - g0) * P, num_idxs_reg=(g1 - g0) * P,
                        elem_size=TW, queue_num=next_q())
                for g0 in range(0, Chi, SUB):
                    g1 = min(g0 + SUB, Chi)
                    nc.gpsimd.dma_gather(
                        out_ap=rows3[:, Clo + g0:Clo + g1, :],
                        in_ap=h_full_b[pr][:],
                        idxs_ap=ihi_t[:, (chi0 + g0) * 8:(chi0 + g1) * 8],
                        num_idxs=(g1 - g0) * P, num_idxs_reg=(g1 - g0) * P,
                        elem_size=TW, queue_num=next_q())
                oh_g = oh_p.tile([P, GMAX * P], fp8, tag="oh", name="oh_g")
                ohT_g = oh_p.tile([P, GMAX * P], fp8, tag="ohT", name="ohT_g")
                if Clo:
                    nc.sync.dma_start(oh_g[:, 0:Clo * P],
                                      ohlo_d[:, clo0 * P:(clo0 + Clo) * P])
                    nc.sync.dma_start(ohT_g[:, 0:Clo * P],
                                      ohTlo_d[:, clo0 * P:(clo0 + Clo) * P])
                if Chi:
                    nc.sync.dma_start(oh_g[:, Clo * P:C * P],
                                      ohhi_d[:, chi0 * P:(chi0 + Chi) * P])
                    nc.sync.dma_start(ohT_g[:, Clo * P:C * P],
                                      ohThi_d[:, chi0 * P:(chi0 + Chi) * P])

                plo, phi = clo0, chi0
                for w in range(w0, w1):
                    ws = slice(w * D, (w + 1) * D)
                    Ka, Kb = k_lo[w], k_hi[w]
                    K = Ka + Kb
                    if K == 0:
                        nc.vector.memset(x_sb[:, ws], 0.0)
                        continue
                    # ad[dst] per edge: one PE matmul per chunk into psum cols
                    ps_ad = psa.tile([P, kmax_w], f32, tag="ad", name="ps_ad")
                    ee_t = sml_p.tile([P, kmax_w], f32, tag="ee", name="ee_t")
                    ci = 0
                    for (Ks, pos0, base) in ((Ka, plo - clo0, 0),
                                             (Kb, phi - chi0, Clo)):
                        for k in range(Ks):
                            tc_ = base + pos0 + k
                            nc.tensor.matmul(
                                ps_ad[:, ci + k:ci + k + 1],
                                lhsT=ohT_g[:, tc_ * P:(tc_ + 1) * P],
                                rhs=ad_t[pr][:, w:w + 1],
                                start=True, stop=True)
                        ci += Ks
                    # s2 = ad[dst] + as[src]; leaky; exp — batched per half
                    ci = 0
                    for (Ks, pos0, base) in ((Ka, plo - clo0, 0),
                                             (Kb, phi - chi0, Clo)):
                        if Ks == 0:
                            continue
                        tb = base + pos0
                        s2 = sml_p.tile([P, kmax_w], f32, tag="s2", name="s2")
                        nc.vector.tensor_tensor(
                            out=s2[:, 0:Ks].unsqueeze(2),
                            in0=ps_ad[:, ci:ci + Ks].unsqueeze(2),
                            in1=rows3[:, tb:tb + Ks, 128:129],
                            op=mybir.AluOpType.add)
                        q = sml_p.tile([P, kmax_w], f32, tag="q", name="q")
                        nc.vector.scalar_tensor_tensor(
                            out=q[:, 0:Ks], in0=s2[:, 0:Ks], scalar=NEG_SLOPE,
                            in1=s2[:, 0:Ks], op0=mybir.AluOpType.mult,
                            op1=mybir.AluOpType.max)
                        nc.scalar.activation(ee_t[:, ci:ci + Ks], q[:, 0:Ks],
                                             mybir.ActivationFunctionType.Exp)
                        ci += Ks
                    # mee = oh * ee (batched per half), then accumulate
                    psum = ps.tile([P, GW], f32, tag="edge", name="psum")
                    oh3 = oh_g[:].rearrange("p (c j) -> p c j", j=P)
                    ci = 0
                    for (Ks, pos0, base) in ((Ka, plo - clo0, 0),
                                             (Kb, phi - chi0, Clo)):
                        if Ks == 0:
                            continue
                        tb = base + pos0
                        mee = mee_p.tile([P, kmax_w * P], bf16, tag="mee",
                                         name="mee")
                        nc.vector.tensor_tensor(
                            out=mee[:].rearrange("p (k j) -> p k j",
                                                 j=P)[:, 0:Ks, :],
                            in0=oh3[:, tb:tb + Ks, :],
                            in1=ee_t[:, ci:ci + Ks].unsqueeze(2)
                                .to_broadcast([P, Ks, P]),
                            op=mybir.AluOpType.mult)
                        for k in range(Ks):
                            nc.tensor.matmul(
                                psum[:], lhsT=mee[:, k * P:(k + 1) * P],
                                rhs=rows3[:, tb + k, 0:GW],
                                start=(ci + k == 0),
                                stop=(ci + k == K - 1))
                        ci += Ks
                    plo += Ka
                    phi += Kb

                    # epilogue: normalize, bias, (relu); EPS keeps padded
                    # window slots (no self-loop) finite
                    se = sml_p.tile([P, 1], f32, tag="se", name="se")
                    nc.vector.tensor_tensor(
                        out=se[:], in0=psum[:, 130:131], in1=eps_t[:],
                        op=mybir.AluOpType.add)
                    rcp = sml_p.tile([P, 1], f32, tag="rcp", name="rcp")
                    nc.vector.reciprocal(rcp[:], se[:])
                    if li < 2:
                        t1 = ups_p.tile([P, D], f32, tag="t1", name="t1")
                        nc.vector.scalar_tensor_tensor(
                            out=t1[:], in0=psum[:, 0:D], scalar=rcp[:, 0:1],
                            in1=b_t[li][:], op0=mybir.AluOpType.mult,
                            op1=mybir.AluOpType.add)
                        nc.vector.tensor_tensor(
                            out=x_sb[:, ws], in0=t1[:],
                            in1=zero_t[:].to_broadcast([P, D]),
                            op=mybir.AluOpType.max)
                        phase_a(li + 1, w, 1 - pr)
                    else:
                        nc.vector.scalar_tensor_tensor(
                            out=x_sb[:, ws], in0=psum[:, 0:D],
                            scalar=rcp[:, 0:1], in1=b_t[li][:],
                            op0=mybir.AluOpType.mult,
                            op1=mybir.AluOpType.add)
            if li < 2:
                ag_half(1 - pr, 0)
                ag_half(1 - pr, 1)

        # ---- global mean pool
        pool_ps = psp.tile([P, D], f32, tag="pool")
        for w in range(NWIN):
            pt = ph_p.tile([P, P], bf16, tag="ppool")
            nc.sync.dma_start(pt[:], pp_d[w * P:(w + 1) * P, :])
            nc.tensor.matmul(pool_ps[:], lhsT=pt[:],
                             rhs=x_sb[:, w * D:(w + 1) * D],
                             start=(w == 0), stop=(w == NWIN - 1))
        po = ups_p.tile([P, D], f32, tag="po")
        nc.vector.tensor_copy(po[:], pool_ps[:])
        nc.sync.dma_start(out_d[:], po[:])

    n_inst = sum(len(bb.instructions) for bb in nc.main_func.blocks)
    print(f"[kernel] instructions: {n_inst}, chunks: {nch_lo}+{nch_hi}, "
          f"groups: {len(groups)}")
    if os.environ.get("KERNEL_TRACE_ONLY") == "1":
        return nc
    t0 = time.time()
    nc.compile()
    print(f"[kernel] bacc compile {time.time() - t0:.1f}s")
    _COMPILED[key] = nc
    return nc


def _install_ntff_hook():
    try:
        import contextlib
        import ctypes
        import types
        if "antenv.axon_hooks" in sys.modules:
            return True
        so = "/opt/axon/libaxon_pjrt.so"
        if not os.path.exists(so):
            return False
        lib = ctypes.CDLL(so)
        if not hasattr(lib, "axon_start_nrt_profile"):
            return False
        lib.axon_start_nrt_profile.argtypes = [ctypes.POINTER(ctypes.c_int64),
                                               ctypes.c_size_t]
        lib.axon_start_nrt_profile.restype = ctypes.c_int64
        lib.axon_stop_nrt_profile.argtypes = [ctypes.c_char_p]
        lib.axon_stop_nrt_profile.restype = ctypes.c_int64

        @contextlib.contextmanager
        def hook(output_dir, device_ids):
            import jax
            jax.devices()
            if device_ids:
                ids = (ctypes.c_int64 * len(device_ids))(*device_ids)
                rc = lib.axon_start_nrt_profile(ids, len(device_ids))
            else:
                rc = lib.axon_start_nrt_profile(None, 0)
            if rc != 0:
                raise RuntimeError(f"axon_start_nrt_profile rc={rc}")
            try:
                yield
            finally:
                lib.axon_stop_nrt_profile(str(output_dir).encode())

        m = types.ModuleType("antenv.axon_hooks")
        m.get_axon_ntff_profile_hook = lambda: hook
        m.set_axon_ntff_profile_hook = lambda h: None
        sys.modules["antenv.axon_hooks"] = m
        import antenv
        antenv.axon_hooks = m
        return True
    except Exception:
        return False


# ---------------------------------------------------------------- entry
def kernel(node_ids, edge_index, batch, emb,
           W1, as1, ad1, b1, W2, as2, ad2, b2, W3, as3, ad3, b3):
    global last_exec_time_ns
    cores, k_lo, k_hi = _prep(np.asarray(node_ids), np.asarray(edge_index),
                              np.asarray(batch))

    def to_bf16(a):
        import jax.numpy as jnp
        return np.asarray(jnp.asarray(np.asarray(a, np.float32), jnp.bfloat16))

    w_all = np.zeros((3, D, HW), np.float32)
    b_bc = np.zeros((3, P, D), np.float32)
    for i, (W, a_s, a_d, b) in enumerate([(W1, as1, ad1, b1), (W2, as2, ad2, b2),
                                          (W3, as3, ad3, b3)]):
        W = np.asarray(W, np.float32)
        w_all[i, :, :D] = W
        w_all[i, :, 128] = W @ np.asarray(a_s, np.float32)
        w_all[i, :, 129] = W @ np.asarray(a_d, np.float32)
        b_bc[i] = np.tile(np.asarray(b, np.float32)[None, :], (P, 1))
    ident = np.eye(P, dtype=np.float32)

    emb_bf = to_bf16(emb)
    in_maps = []
    for c in cores:
        in_maps.append(dict(emb_bf=emb_bf, w_all=to_bf16(w_all), b_bcast=b_bc,
                            ident=to_bf16(ident),
                            nid16=c["nid16"], idx16_lo=c["idx16_lo"],
                            idx16_hi=c["idx16_hi"],
                            oh_lo=c["oh_lo"], oh_hi=c["oh_hi"],
                            ohT_lo=c["ohT_lo"], ohT_hi=c["ohT_hi"],
                            p_pool=to_bf16(c["p_pool"])))

    nc = _build(k_lo, k_hi)
    trace = _install_ntff_hook() and os.environ.get("KERNEL_NO_TRACE") != "1"
    res = run_bass_kernel_spmd(nc, in_maps, list(range(NC)), trace=trace)
    last_exec_time_ns = res.exec_time_ns

    out = np.zeros((N_GRAPHS, D), np.float32)
    for ci, c in enumerate(cores):
        out[c["g0"]:c["g0"] + c["G_c"]] = res.results[ci]["out_pool"][:c["G_c"]]
    return out
